# revision 31
# baseline (speedup 1.0000x reference)
"""DendriticFullyConnected Trainium2 kernel (fp8 DoubleRow version).

Math (per reference):
  x_c  = x[:, :409];  x_nc = x[:, 409:]
  state = sigmoid(x_nc @ W_non.T + b_non) - 1
  cluster = (x_c * coeff) @ W_nmda.T          # coeff = [1,2,...,2,1]
  pre = cluster + state
  out = pre^2 / (0.25 + pre^2)

Strategy: data-parallel over batch on 8 cores (1024 rows each), weights
replicated.  The big "non" contraction (3687 rows + 1 bias row, padded to
3840 = 15 pairs of 256) runs in fp8-e4m3 with perf_mode=DoubleRow: both
operands are quantized to e4m3 on the host (W_non scaled by 64 so its values
are O(1); the 1/64 is folded into the sigmoid's activation scale) and each
matmul contracts 256 rows at 2 fp8 MACs/cell/cycle.  The error lands before
the sigmoid, whose slope (<=0.25) attenuates it: measured rel-l2 ~6.4e-3
vs the 2e-2 budget.  The small "nmda" contraction (409 -> 512 rows, 12% of
FLOPs) feeds the Hill nonlinearity directly (slope up to ~1.3), so it runs
in bf16 (same 1-column/cycle PE rate, weight loads hidden by FWL; f32r
weight loads do NOT overlap and cost ~2.7 us/o-tile extra).  Output is
stored as bf16 and upcast on the host.

Per-core layouts (host-prepared so every device DMA is contiguous):
  xnm [512, 1024] bf16   nmda x, transposed       (4 per-j-tile DMAs)
  xq  [1920, 2048] e4m3  non x, pair-interleaved: row g*128+p, col j*1024+b
                         = x_ncT[(2g+j)*128+p, b]  (15 x 256 KB DMAs)
  wnm [128, 32, 512] bf16  wnm[p,ot,j*128+o]  = WmT[j*128+p,  ot*128+o]
  wq  [128, 32, 3840] e4m3 wq[p,ot,s*128+o]   = WqT[s*128+p, ot*128+o]
                         (one 1 KB/partition resp 3.75 KB/partition DMA
                          per o-tile)

Device loop: nmda phases for the first OT_AHEAD o-tiles run first (they only
need the small xnm cache) to cover the fp8 x-cache fill; then each o-tile
runs its 15 DoubleRow matmuls + 8 bf16 matmuls per batch half, with the
sigmoid/Hill epilogue on ACT/DVE (nmda PSUM->SBUF staging on DVE so the ACT
queue, the epilogue critical path, drains faster at the end).  The x fill
is spread across the Scalar, Sync and GpSimd DMA paths; the last o-tile
runs its batch halves back-to-back so half the final epilogue overlaps the
final matmuls.

Measured model: every matmul flavor (bf16 / f32r / fp8-DR) streams 1 moving
column per cycle at 2.4 GHz, so PE time = 1024 cols x 32 o-tiles x (15 DR
pairs + 4 bf16 k-tiles) ~= 260 us/core; DoubleRow's 2x is entirely from the
doubled contraction per column.  The remainder is a HAM-ramp/fill-bound
~15 us startup and a ~16 us epilogue-drain + NEFF-exit tail.
"""

import numpy as np

B = 8192
IN_F = 4096
OUT_F = 4096
IC = 409                       # clustering synapses
INC = IN_F - IC                # 3687
KD = 0.25                      # Hill k_d = k_a^n = 0.5^2
NCORES = 8
BLOC = B // NCORES             # 1024
KNM_PAD = 512                  # nmda contraction, padded
JNM = KNM_PAD // 128           # 4 nmda k-subtiles
NON_ROWS = INC + 1             # 3688: non rows + bias row
NPAIRS = 15                    # DoubleRow pairs of 256 rows
NON_PAD = NPAIRS * 256         # 3840
OT = OUT_F // 128              # 32 output-row tiles
NBH = BLOC // 512              # 2 batch halves (512 = max matmul out free dim)
WSCALE = 64.0                  # W_non prescale so e4m3 sees O(1) values
OT_AHEAD = 6                   # o-tiles whose nmda phase covers the x fill
KOUT = 4                       # o-tiles in the k-outer startup group

_nc_cache = []


def _build():
    import concourse.bacc as bacc
    import concourse.tile as tile
    import concourse.mybir as mybir

    f32 = mybir.dt.float32
    f32r = mybir.dt.float32r
    fp8 = mybir.dt.float8e4
    bf16 = mybir.dt.bfloat16
    ACT = mybir.ActivationFunctionType
    DR = mybir.MatmulPerfMode.DoubleRow

    nc = bacc.Bacc(None, target_bir_lowering=False)
    xnm = nc.dram_tensor("xnm", [KNM_PAD, BLOC], bf16, kind="ExternalInput")
    xq = nc.dram_tensor("xq", [NPAIRS * 128, 2 * BLOC], fp8, kind="ExternalInput")
    wnm = nc.dram_tensor("wnm", [128, OT, JNM * 128], bf16, kind="ExternalInput")
    wq = nc.dram_tensor("wq", [128, OT, NPAIRS * 256], fp8, kind="ExternalInput")
    outT = nc.dram_tensor("outT", [OUT_F, BLOC], bf16, kind="ExternalOutput")

    with tile.TileContext(nc) as tc:
        with (
            tc.tile_pool(name="xpool", bufs=1) as xpool,
            tc.tile_pool(name="wqp", bufs=6) as wqp,
            tc.tile_pool(name="wnmp", bufs=4) as wnmp,
            tc.tile_pool(name="nmpool", bufs=24) as nmpool,
            tc.tile_pool(name="tmp", bufs=16) as tmp,
            tc.tile_pool(name="psum", bufs=8, space="PSUM") as psum,
        ):
            # ── x caches ────────────────────────────────────────────────
            # nmda x: 4 per-j tiles [128, 1024] bf16 so the first nmda
            # matmul only waits on j-tile 0 (~0.25 MB), not the whole cache.
            # They queue on Sync right after wnm_0 (issued in Phase A below).
            xnm_t = [
                xpool.tile([128, BLOC], bf16, tag=f"xnm{j}", name=f"xnm_{j}")
                for j in range(JNM)
            ]

            def fill_xnm():
                for j in range(JNM):
                    nc.sync.dma_start(xnm_t[j][:], xnm[j * 128 : (j + 1) * 128, :])
            # non x: 15 pair tiles [128, 2, 1024] fp8, spread across three
            # DMA paths so no single queue serializes the 3.75 MB fill:
            #   pairs 0-5  -> Scalar/HWDGE (free until the first epilogue)
            #   pairs 6-10 -> Sync, interleaved between the Phase-A W loads
            #   pairs 11-14-> GpSimd/SWDGE (slowest path ~110 GB/s, so it
            #                 gets the pairs the first sweep needs LAST)
            xq_t = []
            x_sync = []
            for g in range(NPAIRS):
                t = xpool.tile([128, 2, BLOC], fp8, tag=f"xq{g}")
                src = xq[g * 128 : (g + 1) * 128, :].rearrange(
                    "p (j b) -> p j b", j=2
                )
                if g < 6:
                    nc.scalar.dma_start(t[:], src)
                elif g >= 11:
                    nc.gpsimd.dma_start(t[:], src)
                else:
                    x_sync.append((t, src))
                xq_t.append(t)

            def osl(ot):
                return slice(ot * 128, (ot + 1) * 128)

            def load_wnm(ot):
                t = wnmp.tile([128, JNM, 128], bf16, tag="wnm", name=f"wnm_{ot}")
                nc.sync.dma_start(
                    t[:],
                    wnm[:, ot, :].rearrange("p (j o) -> p j o", j=JNM),
                )
                return t

            def load_wq(ot):
                t = wqp.tile([128, 2 * NPAIRS, 128], fp8, tag="wq", name=f"wq_{ot}")
                nc.sync.dma_start(
                    t[:],
                    wq[:, ot, :].rearrange("p (s o) -> p s o", s=2 * NPAIRS),
                )
                return t

            def nmda_phase(ot, wnm_t):
                ps = [
                    psum.tile([128, 512], f32, tag="ps", name=f"psnm_{ot}_{bh}")
                    for bh in range(NBH)
                ]
                for j in range(JNM):
                    for bh in range(NBH):
                        nc.tensor.matmul(
                            ps[bh][:],
                            lhsT=wnm_t[:, j, :],
                            rhs=xnm_t[j][:, bh * 512 : (bh + 1) * 512],
                            start=(j == 0),
                            stop=(j == JNM - 1),
                        )
                nm = []
                for bh in range(NBH):
                    t = nmpool.tile([128, 512], f32, tag="nm", name=f"nm_{ot}_{bh}")
                    # PSUM -> SBUF staging on DVE (add-0), not ACT: the ACT
                    # queue is the epilogue critical path and its end-of-run
                    # backlog is what trails the last matmul.
                    nc.vector.tensor_scalar_add(t[:], ps[bh][:], 0.0)
                    nm.append(t)
                return nm

            def non_sweep(ot, wq_t):
                ps = [
                    psum.tile([128, 512], f32, tag="ps", name=f"psno_{ot}_{bh}")
                    for bh in range(NBH)
                ]
                for g in range(NPAIRS):
                    for bh in range(NBH):
                        nc.tensor.matmul(
                            ps[bh][:],
                            lhsT=wq_t[:, 2 * g : 2 * g + 2, :],
                            rhs=xq_t[g][:, :, bh * 512 : (bh + 1) * 512],
                            start=(g == 0),
                            stop=(g == NPAIRS - 1),
                            perf_mode=DR,
                        )
                return ps

            def epilogue(ot, ps, nm, bhs=None):
                # pre = nm - sigmoid(-(z));  PSUM holds 64*z, so the sigmoid
                # scale is -1/64.  out = pre^2 / (KD + pre^2), stored bf16.
                # Intermediates are bf16: 2x DVE throughput, and the added
                # quantization (~5e-3 rel) fits the 2e-2 budget.  The batch-
                # half chains are interleaved so ACT and DVE overlap.
                if bhs is None:
                    bhs = range(NBH)
                sig, rec, sq, den, ob = {}, {}, {}, {}, {}
                for bh in bhs:
                    sig[bh] = tmp.tile([128, 512], f32, tag="t", name=f"sig_{ot}_{bh}")
                    sq[bh] = tmp.tile([128, 512], f32, tag="t", name=f"sq_{ot}_{bh}")
                    den[bh] = tmp.tile([128, 512], f32, tag="t", name=f"den_{ot}_{bh}")
                    rec[bh] = tmp.tile([128, 512], f32, tag="t", name=f"rec_{ot}_{bh}")
                    ob[bh] = tmp.tile([128, 512], bf16, tag="ob", name=f"ob_{ot}_{bh}")
                for bh in bhs:
                    nc.scalar.activation(
                        sig[bh][:], ps[bh][:], ACT.Sigmoid, scale=-1.0 / WSCALE
                    )
                for bh in bhs:
                    nc.vector.tensor_sub(sig[bh][:], nm[bh][:], sig[bh][:])  # := pre
                for bh in bhs:
                    nc.scalar.activation(sq[bh][:], sig[bh][:], ACT.Square)
                for bh in bhs:
                    nc.vector.tensor_scalar_add(den[bh][:], sq[bh][:], KD)
                for bh in bhs:
                    nc.vector.reciprocal_approx_fast(rec[bh][:], den[bh][:])
                for bh in bhs:
                    nc.vector.tensor_mul(ob[bh][:], sq[bh][:], rec[bh][:])
                for bh in bhs:
                    # ACT is the second HWDGE engine: stores ride its FIFO
                    # where they follow the epilogue anyway, never blocking
                    # the W stream on Sync.
                    bsl = slice(bh * 512, (bh + 1) * 512)
                    nc.scalar.dma_start(outT[osl(ot), bsl], ob[bh][:])

            # ── Phase A: nmda for the first OT_AHEAD o-tiles (covers the
            #    fp8 x-cache fill with PE work that only needs xnm) ──
            # ── Phase A: nmda for o-tiles 0..OT_AHEAD-1 (needs only the
            #    small xnm cache) plus the wq_0..3 prefetch, all while the
            #    fp8 x cache streams in ──
            nm_done = {}
            wq_early = {}
            for ot in range(OT_AHEAD):
                wnm_t = load_wnm(ot)
                if ot == 0:
                    fill_xnm()  # right behind wnm_0: first matmul waits
                    # only on wnm_0 + xnm j-tile 0 (~0.4 MB of DMA)
                if ot < KOUT:
                    wq_early[ot] = load_wq(ot)
                # interleave the sync-side x-pair fill with the Phase-A W loads
                if ot < len(x_sync):
                    t, src = x_sync[ot]
                    nc.sync.dma_start(t[:], src)
                nm_done[ot] = nmda_phase(ot, wnm_t)

            # ── Phase B': k-OUTER non accumulation for o-tiles 0..KOUT-1 on
            #    all 8 PSUM banks.  Each arriving 256 KB x pair unlocks
            #    KOUT*2 DoubleRow matmuls (~1.7 us of PE work per ~0.9 us of
            #    fill), so the PE never idles waiting for the cache tail ──
            ps_ahead = [
                [
                    psum.tile([128, 512], f32, tag="ps", name=f"psB_{ot}_{bh}")
                    for bh in range(NBH)
                ]
                for ot in range(KOUT)
            ]
            for g in range(NPAIRS):
                for ot in range(KOUT):
                    for bh in range(NBH):
                        nc.tensor.matmul(
                            ps_ahead[ot][bh][:],
                            lhsT=wq_early[ot][:, 2 * g : 2 * g + 2, :],
                            rhs=xq_t[g][:, :, bh * 512 : (bh + 1) * 512],
                            start=(g == 0),
                            stop=(g == NPAIRS - 1),
                            perf_mode=DR,
                        )
            for ot in range(KOUT):
                epilogue(ot, ps_ahead[ot], nm_done.pop(ot))
                nm_done[ot + OT_AHEAD] = nmda_phase(
                    ot + OT_AHEAD, load_wnm(ot + OT_AHEAD)
                )

            # ── Phase B: per-o-tile non sweep + pipelined nmda(ot+AHEAD).
            #    The nmda copies are issued BEFORE the epilogue so the
            #    nmda PSUM banks free promptly ──
            for ot in range(KOUT, OT - 1):
                wq_t = load_wq(ot)
                ps = non_sweep(ot, wq_t)
                nx = ot + OT_AHEAD
                if KOUT + OT_AHEAD <= nx < OT:
                    nm_done[nx] = nmda_phase(nx, load_wnm(nx))
                epilogue(ot, ps, nm_done.pop(ot))

            # ── Last o-tile: run the batch halves back-to-back so bh0's
            #    epilogue overlaps bh1's sweep instead of trailing the
            #    final matmul ──
            ot = OT - 1
            wq_t = load_wq(ot)
            nm = nm_done.pop(ot)
            ps = [
                psum.tile([128, 512], f32, tag="ps", name=f"pslast_{bh}")
                for bh in range(NBH)
            ]
            for bh in range(NBH):
                for g in range(NPAIRS):
                    nc.tensor.matmul(
                        ps[bh][:],
                        lhsT=wq_t[:, 2 * g : 2 * g + 2, :],
                        rhs=xq_t[g][:, :, bh * 512 : (bh + 1) * 512],
                        start=(g == 0),
                        stop=(g == NPAIRS - 1),
                        perf_mode=DR,
                    )
                epilogue(ot, ps, nm, bhs=[bh])
    nc.compile()
    return nc


def _warmup():
    """Tiny throwaway NEFF run: the first execution after session start
    occasionally dies with NRT_EXEC_UNIT_UNRECOVERABLE; absorb that here."""
    import concourse.bacc as bacc
    import concourse.tile as tile
    import concourse.mybir as mybir
    from concourse.bass_utils import run_bass_kernel_spmd

    nc = bacc.Bacc(None, target_bir_lowering=False)
    a = nc.dram_tensor("a", [128, 128], mybir.dt.float32, kind="ExternalInput")
    b = nc.dram_tensor("b", [128, 128], mybir.dt.float32, kind="ExternalOutput")
    with tile.TileContext(nc) as tc:
        with tc.tile_pool(name="p", bufs=1) as pool:
            t = pool.tile([128, 128], mybir.dt.float32)
            nc.sync.dma_start(t[:], a[:])
            nc.sync.dma_start(b[:], t[:])
    nc.compile()
    ins = [{"a": np.zeros((128, 128), np.float32)} for _ in range(NCORES)]
    for _ in range(3):
        try:
            run_bass_kernel_spmd(nc, ins, core_ids=list(range(NCORES)))
            return
        except Exception:
            continue


def kernel(x, W_nmda, W_non, b_non):
    import ml_dtypes
    from concourse.bass_utils import run_bass_kernel_spmd

    e4 = ml_dtypes.float8_e4m3  # TRN fp8e4-compatible for |v| <= 240

    x = np.asarray(x, dtype=np.float32)
    W_nmda = np.asarray(W_nmda, dtype=np.float32)
    W_non = np.asarray(W_non, dtype=np.float32)
    b_non = np.asarray(b_non, dtype=np.float32)

    coeff = np.full((IC,), 2.0, dtype=np.float32)
    coeff[0] = 1.0
    coeff[-1] = 1.0

    # nmda weights (bf16, coeff folded in): wnm[p, ot, j*128+o]
    Wm = np.zeros((KNM_PAD, OUT_F), dtype=np.float32)
    Wm[:IC] = (W_nmda * coeff[None, :]).T
    wnm_h = np.ascontiguousarray(
        Wm.astype(ml_dtypes.bfloat16)
        .reshape(JNM, 128, OT, 128)
        .transpose(1, 2, 0, 3)
        .reshape(128, OT, JNM * 128)
    )

    # non weights + bias row, scaled and quantized: wq[p, ot, s*128+o]
    Wq = np.zeros((NON_PAD, OUT_F), dtype=np.float32)
    Wq[:INC] = W_non.T * WSCALE
    Wq[INC] = b_non * WSCALE
    wq_h = np.ascontiguousarray(
        Wq.astype(e4)
        .reshape(NPAIRS, 2, 128, OT, 128)
        .transpose(2, 3, 0, 1, 4)
        .reshape(128, OT, NPAIRS * 256)
    )

    # x, transposed and split
    xnm_full = np.zeros((KNM_PAD, B), dtype=ml_dtypes.bfloat16)
    xnm_full[:IC] = x[:, :IC].T.astype(ml_dtypes.bfloat16)
    Xn = np.zeros((NON_PAD, B), dtype=np.float32)
    Xn[:INC] = x[:, IC:].T
    Xn[INC] = 1.0  # bias row
    Xn8 = Xn.astype(e4)

    in_maps = []
    for c in range(NCORES):
        sl = slice(c * BLOC, (c + 1) * BLOC)
        xq_c = np.ascontiguousarray(
            Xn8[:, sl]
            .reshape(NPAIRS, 2, 128, BLOC)
            .transpose(0, 2, 1, 3)
            .reshape(NPAIRS * 128, 2 * BLOC)
        )
        in_maps.append(
            {
                "xnm": np.ascontiguousarray(xnm_full[:, sl]),
                "xq": xq_c,
                "wnm": wnm_h,
                "wq": wq_h,
            }
        )

    if not _nc_cache:
        _warmup()
        _nc_cache.append(_build())
    nc = _nc_cache[0]

    res = None
    last_exc = None
    for _attempt in range(3):
        try:
            res = run_bass_kernel_spmd(nc, in_maps, core_ids=list(range(NCORES)))
            break
        except Exception as e:  # transient device errors (e.g. first-run NRT hiccup)
            last_exc = e
    if res is None:
        raise last_exc

    global LAST_RESULT
    LAST_RESULT = res

    out = np.empty((B, OUT_F), dtype=np.float32)
    for c in range(NCORES):
        out[c * BLOC : (c + 1) * BLOC] = res.results[c]["outT"].astype(np.float32).T
    return out


LAST_RESULT = None


# revision 34
# speedup vs baseline: 1.0103x; 1.0103x over previous
"""DendriticFullyConnected Trainium2 kernel (fp8 DoubleRow version).

Math (per reference):
  x_c  = x[:, :409];  x_nc = x[:, 409:]
  state = sigmoid(x_nc @ W_non.T + b_non) - 1
  cluster = (x_c * coeff) @ W_nmda.T          # coeff = [1,2,...,2,1]
  pre = cluster + state
  out = pre^2 / (0.25 + pre^2)

Strategy: data-parallel over batch on 8 cores (1024 rows each), weights
replicated.  The big "non" contraction (3687 rows + 1 bias row, padded to
3840 = 15 pairs of 256) runs in fp8-e4m3 with perf_mode=DoubleRow: both
operands are quantized to e4m3 on the host (W_non scaled by 64 so its values
are O(1); the 1/64 is folded into the sigmoid's activation scale) and each
matmul contracts 256 rows at 2 fp8 MACs/cell/cycle.  The error lands before
the sigmoid, whose slope (<=0.25) attenuates it: measured rel-l2 ~6.4e-3
vs the 2e-2 budget.  The small "nmda" contraction (409 -> 512 rows, 12% of
FLOPs) feeds the Hill nonlinearity directly (slope up to ~1.3), so it runs
in bf16 (same 1-column/cycle PE rate, weight loads hidden by FWL; f32r
weight loads do NOT overlap and cost ~2.7 us/o-tile extra).  Output is
stored as bf16 and upcast on the host.

Per-core layouts (host-prepared so every device DMA is contiguous):
  xnm [512, 1024] bf16   nmda x, transposed       (4 per-j-tile DMAs)
  xq  [1920, 2048] e4m3  non x, pair-interleaved: row g*128+p, col j*1024+b
                         = x_ncT[(2g+j)*128+p, b]  (15 x 256 KB DMAs)
  wnm [128, 32, 512] bf16  wnm[p,ot,j*128+o]  = WmT[j*128+p,  ot*128+o]
  wq  [128, 32, 3840] e4m3 wq[p,ot,s*128+o]   = WqT[s*128+p, ot*128+o]
                         (one 1 KB/partition resp 3.75 KB/partition DMA
                          per o-tile)

Device loop: nmda phases for the first OT_AHEAD o-tiles run first (they only
need the small xnm cache) to cover the fp8 x-cache fill; then each o-tile
runs its 15 DoubleRow matmuls + 8 bf16 matmuls per batch half, with the
sigmoid/Hill epilogue on ACT/DVE (nmda PSUM->SBUF staging on DVE so the ACT
queue, the epilogue critical path, drains faster at the end).  The x fill
is spread across the Scalar, Sync and GpSimd DMA paths; the last o-tile
runs its batch halves back-to-back so half the final epilogue overlaps the
final matmuls.

Measured model: every matmul flavor (bf16 / f32r / fp8-DR) streams 1 moving
column per cycle at 2.4 GHz, so PE time = 1024 cols x 32 o-tiles x (15 DR
pairs + 4 bf16 k-tiles) ~= 260 us/core; DoubleRow's 2x is entirely from the
doubled contraction per column.  The remainder is a HAM-ramp/fill-bound
~15 us startup and a ~16 us epilogue-drain + NEFF-exit tail.
"""

import numpy as np

B = 8192
IN_F = 4096
OUT_F = 4096
IC = 409                       # clustering synapses
INC = IN_F - IC                # 3687
KD = 0.25                      # Hill k_d = k_a^n = 0.5^2
NCORES = 8
BLOC = B // NCORES             # 1024
KNM_PAD = 512                  # nmda contraction, padded
JNM = KNM_PAD // 128           # 4 nmda k-subtiles
NON_ROWS = INC + 1             # 3688: non rows + bias row
NPAIRS = 15                    # DoubleRow pairs of 256 rows
NON_PAD = NPAIRS * 256         # 3840
OT = OUT_F // 128              # 32 output-row tiles
NBH = BLOC // 512              # 2 batch halves (512 = max matmul out free dim)
WSCALE = 64.0                  # W_non prescale so e4m3 sees O(1) values
OT_AHEAD = 6                   # o-tiles whose nmda phase covers the x fill

_nc_cache = []


def _build():
    import concourse.bacc as bacc
    import concourse.tile as tile
    import concourse.mybir as mybir

    f32 = mybir.dt.float32
    f32r = mybir.dt.float32r
    fp8 = mybir.dt.float8e4
    bf16 = mybir.dt.bfloat16
    ACT = mybir.ActivationFunctionType
    DR = mybir.MatmulPerfMode.DoubleRow

    nc = bacc.Bacc(None, target_bir_lowering=False)
    xnm = nc.dram_tensor("xnm", [KNM_PAD, BLOC], bf16, kind="ExternalInput")
    xq = nc.dram_tensor("xq", [NPAIRS * 128, 2 * BLOC], fp8, kind="ExternalInput")
    wnm = nc.dram_tensor("wnm", [128, OT, JNM * 128], bf16, kind="ExternalInput")
    wq = nc.dram_tensor("wq", [128, OT, NPAIRS * 256], fp8, kind="ExternalInput")
    outT = nc.dram_tensor("outT", [OUT_F, BLOC], bf16, kind="ExternalOutput")

    with tile.TileContext(nc) as tc:
        with (
            tc.tile_pool(name="xpool", bufs=1) as xpool,
            tc.tile_pool(name="wqp", bufs=3) as wqp,
            tc.tile_pool(name="wnmp", bufs=3) as wnmp,
            tc.tile_pool(name="nmpool", bufs=24) as nmpool,
            tc.tile_pool(name="tmp", bufs=16) as tmp,
            tc.tile_pool(name="psum", bufs=8, space="PSUM") as psum,
        ):
            # ── x caches ────────────────────────────────────────────────
            # nmda x: 4 per-j tiles [128, 1024] bf16 so the first nmda
            # matmul only waits on j-tile 0 (~0.25 MB), not the whole cache.
            # They queue on Sync right after wnm_0 (issued in Phase A below).
            xnm_t = [
                xpool.tile([128, BLOC], bf16, tag=f"xnm{j}", name=f"xnm_{j}")
                for j in range(JNM)
            ]

            def fill_xnm():
                for j in range(JNM):
                    nc.sync.dma_start(xnm_t[j][:], xnm[j * 128 : (j + 1) * 128, :])
            # non x: 15 pair tiles [128, 2, 1024] fp8, spread across three
            # DMA paths so no single queue serializes the 3.75 MB fill:
            #   pairs 0-5  -> Scalar/HWDGE (free until the first epilogue)
            #   pairs 6-10 -> Sync, interleaved between the Phase-A W loads
            #   pairs 11-14-> GpSimd/SWDGE (slowest path ~110 GB/s, so it
            #                 gets the pairs the first sweep needs LAST)
            xq_t = []
            x_sync = []
            for g in range(NPAIRS):
                t = xpool.tile([128, 2, BLOC], fp8, tag=f"xq{g}")
                src = xq[g * 128 : (g + 1) * 128, :].rearrange(
                    "p (j b) -> p j b", j=2
                )
                if g < 6:
                    nc.scalar.dma_start(t[:], src)
                elif g >= 11:
                    nc.gpsimd.dma_start(t[:], src)
                else:
                    x_sync.append((t, src))
                xq_t.append(t)

            def osl(ot):
                return slice(ot * 128, (ot + 1) * 128)

            def load_wnm(ot):
                t = wnmp.tile([128, JNM, 128], bf16, tag="wnm", name=f"wnm_{ot}")
                nc.sync.dma_start(
                    t[:],
                    wnm[:, ot, :].rearrange("p (j o) -> p j o", j=JNM),
                )
                return t

            def load_wq(ot):
                t = wqp.tile([128, 2 * NPAIRS, 128], fp8, tag="wq", name=f"wq_{ot}")
                nc.sync.dma_start(
                    t[:],
                    wq[:, ot, :].rearrange("p (s o) -> p s o", s=2 * NPAIRS),
                )
                return t

            def nmda_phase(ot, wnm_t):
                ps = [
                    psum.tile([128, 512], f32, tag="ps", name=f"psnm_{ot}_{bh}")
                    for bh in range(NBH)
                ]
                for j in range(JNM):
                    for bh in range(NBH):
                        nc.tensor.matmul(
                            ps[bh][:],
                            lhsT=wnm_t[:, j, :],
                            rhs=xnm_t[j][:, bh * 512 : (bh + 1) * 512],
                            start=(j == 0),
                            stop=(j == JNM - 1),
                        )
                nm = []
                for bh in range(NBH):
                    t = nmpool.tile([128, 512], f32, tag="nm", name=f"nm_{ot}_{bh}")
                    # PSUM -> SBUF staging on DVE (add-0), not ACT: the ACT
                    # queue is the epilogue critical path and its end-of-run
                    # backlog is what trails the last matmul.
                    nc.vector.tensor_scalar_add(t[:], ps[bh][:], 0.0)
                    nm.append(t)
                return nm

            def non_sweep(ot, wq_t):
                ps = [
                    psum.tile([128, 512], f32, tag="ps", name=f"psno_{ot}_{bh}")
                    for bh in range(NBH)
                ]
                for g in range(NPAIRS):
                    for bh in range(NBH):
                        nc.tensor.matmul(
                            ps[bh][:],
                            lhsT=wq_t[:, 2 * g : 2 * g + 2, :],
                            rhs=xq_t[g][:, :, bh * 512 : (bh + 1) * 512],
                            start=(g == 0),
                            stop=(g == NPAIRS - 1),
                            perf_mode=DR,
                        )
                return ps

            def epilogue(ot, ps, nm, bhs=None):
                # pre = nm - sigmoid(-(z));  PSUM holds 64*z, so the sigmoid
                # scale is -1/64.  out = pre^2 / (KD + pre^2), stored bf16.
                # Intermediates are bf16: 2x DVE throughput, and the added
                # quantization (~5e-3 rel) fits the 2e-2 budget.  The batch-
                # half chains are interleaved so ACT and DVE overlap.
                if bhs is None:
                    bhs = range(NBH)
                sig, rec, sq, den, ob = {}, {}, {}, {}, {}
                for bh in bhs:
                    sig[bh] = tmp.tile([128, 512], f32, tag="t", name=f"sig_{ot}_{bh}")
                    sq[bh] = tmp.tile([128, 512], f32, tag="t", name=f"sq_{ot}_{bh}")
                    den[bh] = tmp.tile([128, 512], f32, tag="t", name=f"den_{ot}_{bh}")
                    rec[bh] = tmp.tile([128, 512], f32, tag="t", name=f"rec_{ot}_{bh}")
                    ob[bh] = tmp.tile([128, 512], bf16, tag="ob", name=f"ob_{ot}_{bh}")
                for bh in bhs:
                    nc.scalar.activation(
                        sig[bh][:], ps[bh][:], ACT.Sigmoid, scale=-1.0 / WSCALE
                    )
                for bh in bhs:
                    nc.vector.tensor_sub(sig[bh][:], nm[bh][:], sig[bh][:])  # := pre
                for bh in bhs:
                    nc.scalar.activation(sq[bh][:], sig[bh][:], ACT.Square)
                for bh in bhs:
                    nc.vector.tensor_scalar_add(den[bh][:], sq[bh][:], KD)
                for bh in bhs:
                    nc.vector.reciprocal_approx_fast(rec[bh][:], den[bh][:])
                for bh in bhs:
                    nc.vector.tensor_mul(ob[bh][:], sq[bh][:], rec[bh][:])
                for bh in bhs:
                    # ACT is the second HWDGE engine: stores ride its FIFO
                    # where they follow the epilogue anyway, never blocking
                    # the W stream on Sync.
                    bsl = slice(bh * 512, (bh + 1) * 512)
                    nc.scalar.dma_start(outT[osl(ot), bsl], ob[bh][:])

            # ── Phase A: nmda for the first OT_AHEAD o-tiles (covers the
            #    fp8 x-cache fill with PE work that only needs xnm) ──
            # ── Phase A: nmda for o-tiles 0..OT_AHEAD-1 (needs only the
            #    small xnm cache), covering the fp8 x-cache fill.  A k-outer
            #    startup phase (first KOUT o-tiles accumulating pair-by-pair
            #    on all 8 PSUM banks) was tried and measured SLOWER: the
            #    startup is aggregate-DMA-bound, and preloading wq_0..3
            #    earlier only delays Phase A's own inputs ──
            nm_done = {}
            for ot in range(OT_AHEAD):
                wnm_t = load_wnm(ot)
                if ot == 0:
                    fill_xnm()  # right behind wnm_0: first matmul waits
                    # only on wnm_0 + xnm j-tile 0 (~0.4 MB of DMA)
                # interleave the sync-side x-pair fill with the Phase-A W loads
                if ot < len(x_sync):
                    t, src = x_sync[ot]
                    nc.sync.dma_start(t[:], src)
                nm_done[ot] = nmda_phase(ot, wnm_t)

            # ── Phase B: per-o-tile non sweep + pipelined nmda(ot+AHEAD).
            #    The nmda copies are issued BEFORE the epilogue so the
            #    nmda PSUM banks free promptly ──
            for ot in range(OT - 1):
                wq_t = load_wq(ot)
                ps = non_sweep(ot, wq_t)
                if ot + OT_AHEAD < OT:
                    nm_done[ot + OT_AHEAD] = nmda_phase(
                        ot + OT_AHEAD, load_wnm(ot + OT_AHEAD)
                    )
                epilogue(ot, ps, nm_done.pop(ot))

            # ── Last o-tile: run the batch halves back-to-back so bh0's
            #    epilogue overlaps bh1's sweep instead of trailing the
            #    final matmul ──
            ot = OT - 1
            wq_t = load_wq(ot)
            nm = nm_done.pop(ot)
            ps = [
                psum.tile([128, 512], f32, tag="ps", name=f"pslast_{bh}")
                for bh in range(NBH)
            ]
            for bh in range(NBH):
                for g in range(NPAIRS):
                    nc.tensor.matmul(
                        ps[bh][:],
                        lhsT=wq_t[:, 2 * g : 2 * g + 2, :],
                        rhs=xq_t[g][:, :, bh * 512 : (bh + 1) * 512],
                        start=(g == 0),
                        stop=(g == NPAIRS - 1),
                        perf_mode=DR,
                    )
                epilogue(ot, ps, nm, bhs=[bh])
    nc.compile()
    return nc


def _warmup():
    """Tiny throwaway NEFF run: the first execution after session start
    occasionally dies with NRT_EXEC_UNIT_UNRECOVERABLE; absorb that here."""
    import concourse.bacc as bacc
    import concourse.tile as tile
    import concourse.mybir as mybir
    from concourse.bass_utils import run_bass_kernel_spmd

    nc = bacc.Bacc(None, target_bir_lowering=False)
    a = nc.dram_tensor("a", [128, 128], mybir.dt.float32, kind="ExternalInput")
    b = nc.dram_tensor("b", [128, 128], mybir.dt.float32, kind="ExternalOutput")
    with tile.TileContext(nc) as tc:
        with tc.tile_pool(name="p", bufs=1) as pool:
            t = pool.tile([128, 128], mybir.dt.float32)
            nc.sync.dma_start(t[:], a[:])
            nc.sync.dma_start(b[:], t[:])
    nc.compile()
    ins = [{"a": np.zeros((128, 128), np.float32)} for _ in range(NCORES)]
    for _ in range(3):
        try:
            run_bass_kernel_spmd(nc, ins, core_ids=list(range(NCORES)))
            return
        except Exception:
            continue


def kernel(x, W_nmda, W_non, b_non):
    import ml_dtypes
    from concourse.bass_utils import run_bass_kernel_spmd

    e4 = ml_dtypes.float8_e4m3  # TRN fp8e4-compatible for |v| <= 240

    x = np.asarray(x, dtype=np.float32)
    W_nmda = np.asarray(W_nmda, dtype=np.float32)
    W_non = np.asarray(W_non, dtype=np.float32)
    b_non = np.asarray(b_non, dtype=np.float32)

    coeff = np.full((IC,), 2.0, dtype=np.float32)
    coeff[0] = 1.0
    coeff[-1] = 1.0

    # nmda weights (bf16, coeff folded in): wnm[p, ot, j*128+o]
    Wm = np.zeros((KNM_PAD, OUT_F), dtype=np.float32)
    Wm[:IC] = (W_nmda * coeff[None, :]).T
    wnm_h = np.ascontiguousarray(
        Wm.astype(ml_dtypes.bfloat16)
        .reshape(JNM, 128, OT, 128)
        .transpose(1, 2, 0, 3)
        .reshape(128, OT, JNM * 128)
    )

    # non weights + bias row, scaled and quantized: wq[p, ot, s*128+o]
    Wq = np.zeros((NON_PAD, OUT_F), dtype=np.float32)
    Wq[:INC] = W_non.T * WSCALE
    Wq[INC] = b_non * WSCALE
    wq_h = np.ascontiguousarray(
        Wq.astype(e4)
        .reshape(NPAIRS, 2, 128, OT, 128)
        .transpose(2, 3, 0, 1, 4)
        .reshape(128, OT, NPAIRS * 256)
    )

    # x, transposed and split
    xnm_full = np.zeros((KNM_PAD, B), dtype=ml_dtypes.bfloat16)
    xnm_full[:IC] = x[:, :IC].T.astype(ml_dtypes.bfloat16)
    Xn = np.zeros((NON_PAD, B), dtype=np.float32)
    Xn[:INC] = x[:, IC:].T
    Xn[INC] = 1.0  # bias row
    Xn8 = Xn.astype(e4)

    in_maps = []
    for c in range(NCORES):
        sl = slice(c * BLOC, (c + 1) * BLOC)
        xq_c = np.ascontiguousarray(
            Xn8[:, sl]
            .reshape(NPAIRS, 2, 128, BLOC)
            .transpose(0, 2, 1, 3)
            .reshape(NPAIRS * 128, 2 * BLOC)
        )
        in_maps.append(
            {
                "xnm": np.ascontiguousarray(xnm_full[:, sl]),
                "xq": xq_c,
                "wnm": wnm_h,
                "wq": wq_h,
            }
        )

    if not _nc_cache:
        _warmup()
        _nc_cache.append(_build())
    nc = _nc_cache[0]

    res = None
    last_exc = None
    for _attempt in range(3):
        try:
            res = run_bass_kernel_spmd(nc, in_maps, core_ids=list(range(NCORES)))
            break
        except Exception as e:  # transient device errors (e.g. first-run NRT hiccup)
            last_exc = e
    if res is None:
        raise last_exc

    global LAST_RESULT
    LAST_RESULT = res

    out = np.empty((B, OUT_F), dtype=np.float32)
    for c in range(NCORES):
        out[c * BLOC : (c + 1) * BLOC] = res.results[c]["outT"].astype(np.float32).T
    return out


LAST_RESULT = None
